# revision 29
# baseline (speedup 1.0000x reference)
"""GQA attention (B=2, L=2048, D=2048, Hq=32, Hkv=8, hd=64) on 8 TRN2 cores.

Tensor-parallel over heads: core c owns q heads 4c..4c+3 and kv head c.
Each core computes a partial output (wo input-dim shard); host sums partials.

Per-core layouts (feature-on-partition, "transposed" convention):
  xT      [2048, 4096]   x transposed (shared by all cores), bf16
  wq_t    [2048, 256]    wq shard rows, per-head [even|odd] dim perm, T, bf16
  wk_t    [2048, 64]     wk shard rows, [even|odd] perm, transposed, bf16
  wv_t    [2048, 64]     wv shard rows (natural order), transposed, bf16
  wo_t    [256, 2048]    wo columns shard, transposed, bf16
  cosb/sinb [128, 2048]  host-computed RoPE tables (sign baked into sin), bf16
  tri     [128, 128]     causal triangle mask (query col q attends key row r
                         iff r <= q), bf16
  outT    [2048, 4096]   partial output, transposed, bf16 (host: sum, T)

Kernel phases: QKV projection -> RoPE -> flash-style attention (S.T layout,
no-max softmax via ones-augmented V matmul for the denominator) -> out proj.
bf16 matmul pipeline; even/odd heads of a pair are row-packed (K=64 tiles at
base partitions 0/64) into one [128,1024] PSUM tile -> single batched exp per
(jc, head-pair). Diagonal-block matmuls are narrowed to the causally valid
column range; fully-masked columns are never written or read.
"""
import ml_dtypes
import numpy as np
from contextlib import ExitStack

import concourse.bass as bass
import concourse.mybir as mybir
import concourse.tile as tile
from concourse import bacc
from concourse.bass_utils import run_bass_kernel_spmd

F32 = mybir.dt.float32
F32R = mybir.dt.float32r
BF16 = mybir.dt.bfloat16
I32 = mybir.dt.int32
AF = mybir.ActivationFunctionType
ALU = mybir.AluOpType

B, L, D = 2, 2048, 2048
HQ, HKV, HD = 32, 8, 64
NCORES = 8
HL = HQ // NCORES          # 4 q heads per core
DQ = HL * HD               # 256 local q features
T = B * L                  # 4096 tokens
NB = 512                   # token block
NT = T // NB               # 8 token blocks
KC = D // 128              # 16 contraction chunks
ROPE_BASE = 10000.0
SCALE = 1.0 / np.sqrt(HD)

_CACHE = {}


def _build_module():
    nc = bacc.Bacc("TRN2", target_bir_lowering=False, debug=False,
                   num_devices=NCORES)

    d_xT = nc.dram_tensor("xT", [D, T], BF16, kind="ExternalInput").ap()
    d_wq = nc.dram_tensor("wq_t", [D, DQ], BF16, kind="ExternalInput").ap()
    d_wkv = nc.dram_tensor("wkv_t", [D, 128], BF16, kind="ExternalInput").ap()
    d_wo = nc.dram_tensor("wo_t", [DQ, D], BF16, kind="ExternalInput").ap()
    d_ident = nc.dram_tensor("ident", [64, 64], BF16, kind="ExternalInput").ap()
    d_cos = nc.dram_tensor("cosb", [128, L], BF16, kind="ExternalInput").ap()
    d_sin = nc.dram_tensor("sinb", [128, L], BF16, kind="ExternalInput").ap()
    d_tri = nc.dram_tensor("tri", [128, 128], BF16, kind="ExternalInput").ap()
    d_onesv = nc.dram_tensor("onesv", [128, 32 * 65], BF16, kind="ExternalInput").ap()
    d_ones2 = nc.dram_tensor("ones2", [1, 128], F32R, kind="ExternalInput").ap()
    d_out = nc.dram_tensor("outT", [D, T], BF16, kind="ExternalOutput").ap()

    with tile.TileContext(nc) as tc, ExitStack() as ctx, \
         nc.allow_low_precision(reason="bf16 matmul pipeline"):
        _kernel(tc, ctx, d_xT, d_wq, d_wkv, d_wo, d_cos, d_sin, d_tri,
                d_onesv, d_ones2, d_ident, d_out)

    nc.compile()
    return nc


def _kernel(tc, ctx, d_xT, d_wq, d_wkv, d_wo, d_cos, d_sin, d_tri,
            d_onesv, d_ones2, d_ident, d_out, dump=None):
    nc = tc.nc

    wpool = ctx.enter_context(tc.tile_pool(name="weights", bufs=1))
    spool = ctx.enter_context(tc.tile_pool(name="state", bufs=1))

    # ---------------- persistent SBUF tensors ----------------
    # wq/wkv chunk DMAs are interleaved with the first x-block loads inside
    # the projection loop (same queue -> first matmul starts at ~1.5us).
    # Everything not needed immediately goes on the GpSimd-triggered queue.
    wqT = wpool.tile([128, KC * DQ], BF16, tag="wqT")      # 8KB/part
    wkvT = wpool.tile([128, KC * 128], BF16, tag="wkvT")   # 4KB
    woT = wpool.tile([128, 2 * D], BF16, tag="woT")        # 8KB

    c128 = spool.tile([128, L], BF16, tag="c128")          # 4KB
    s128 = spool.tile([128, L], BF16, tag="s128")          # 4KB
    nc.gpsimd.dma_start(c128[:], d_cos[:])
    nc.gpsimd.dma_start(s128[:], d_sin[:])
    tri = spool.tile([128, 128], BF16, tag="tri")
    nc.gpsimd.dma_start(tri[:], d_tri[:])
    ident = wpool.tile([64, 64], BF16, tag="ident")
    nc.gpsimd.dma_start(ident[:], d_ident[:])
    # ones2 = [0]*64 + [1]*64: slice [64:128] is an all-ones [1,64]
    # stationary; the full [1,128] broadcasts only into rows 64:128
    ones2 = wpool.tile([1, 128], F32R, tag="ones2")
    nc.gpsimd.dma_start(ones2[:], d_ones2[:])

    # v natural layout + interleaved ones column: chunk ch = cols [65ch, 65ch+65)
    vA = spool.tile([128, 32 * 65], BF16, tag="vA")        # 4.2KB
    # ones columns (cols 65ch+64) via one contiguous full-tile DMA; data
    # cols are zero-filled here and overwritten by the v evictions
    nc.gpsimd.dma_start(vA[:], d_onesv[:])
    for kc2 in range(2):
        nc.gpsimd.dma_start(woT[:, kc2 * D:(kc2 + 1) * D],
                            d_wo[kc2 * 128:(kc2 + 1) * 128, :])

    # qT: [128, HL/2 * T]; head pair p on partitions (even head rows 0:64,
    # odd head rows 64:128); within a head [even dims | odd dims].
    qT = spool.tile([128, 2 * T], BF16, tag="qT")          # 16KB
    # kT duplicated on partitions 64:128 so odd-head matmuls row-tile at 64.
    kT = spool.tile([128, T], BF16, tag="kT")              # 8KB
    # attention output, transposed: head pair tiles, b-major columns
    atP = [spool.tile([128, T], BF16, tag=f"atP{p}", name=f"atP{p}") for p in range(2)]  # 16KB

    # ---------------- phase 1.5: RoPE (all-bf16 DVE ops) -------------------
    tpool = ctx.enter_context(tc.tile_pool(name="tmp", bufs=2))

    RW = 2 * NB  # rope over two token blocks at once (amortize DVE op cost)

    def rope(dst, cols, l0):
        # y = x*cos + swap(x)*sgn*sin, swap = exchange 32-row halves per head
        u = tpool.tile([128, RW], BF16, tag="ropeU")
        w = tpool.tile([128, RW], BF16, tag="ropeW")
        xsw = tpool.tile([128, RW], BF16, tag="ropeX")
        for band in range(4):
            r0, r1 = band * 32, (band ^ 1) * 32
            nc.vector.tensor_copy(xsw[r0:r0 + 32, :], dst[r1:r1 + 32, cols])
        nc.vector.tensor_mul(u[:], dst[:, cols], c128[:, l0:l0 + RW])
        nc.vector.tensor_mul(w[:], xsw[:], s128[:, l0:l0 + RW])
        nc.vector.tensor_add(dst[:, cols], u[:], w[:])

    def rope_nt(nt):
        # called on odd nt: ropes token blocks nt-1 and nt together
        b, l0 = nt // 4, (nt % 4 - 1) * NB
        for p in range(2):
            c0 = p * T + b * L + l0
            rope(qT, slice(c0, c0 + RW), l0)
        rope(kT, slice(b * L + l0, b * L + l0 + RW), l0)

    # ---------------- phase 1: QKV projections ----------------------------
    # k and v projections share one [128,128] stationary (cols 0:64 = wk,
    # 64:128 = wv); v comes out feature-major and is PE-transposed into vA.
    with tc.tile_pool(name="xs", bufs=6) as xs, \
         tc.tile_pool(name="vstage", bufs=2) as vs, \
         tc.tile_pool(name="pproj", bufs=2, space="PSUM") as pq, \
         tc.tile_pool(name="pprojkv", bufs=2, space="PSUM") as pkv, \
         tc.tile_pool(name="ptr", bufs=2, space="PSUM") as ptr:
        for nt in range(NT):
            psq = [pq.tile([128, NB], F32, tag=f"psq{i}", name=f"psq{nt}_{i}") for i in range(2)]
            pskv = pkv.tile([128, NB], F32, tag="pskv", name=f"pskv{nt}")
            for kc in range(KC):
                xk = xs.tile([128, NB], BF16, tag="xk")
                if nt == 0:
                    nc.sync.dma_start(wqT[:, kc * DQ:(kc + 1) * DQ],
                                      d_wq[kc * 128:(kc + 1) * 128, :])
                    nc.sync.dma_start(wkvT[:, kc * 128:(kc + 1) * 128],
                                      d_wkv[kc * 128:(kc + 1) * 128, :])
                nc.sync.dma_start(
                    xk[:], d_xT[kc * 128:(kc + 1) * 128, nt * NB:(nt + 1) * NB])
                st, sp = kc == 0, kc == KC - 1
                for p in range(2):
                    nc.tensor.matmul(
                        psq[p][:], wqT[:, kc * DQ + p * 128: kc * DQ + (p + 1) * 128],
                        xk[:], start=st, stop=sp)
                nc.tensor.matmul(pskv[:], wkvT[:, kc * 128:(kc + 1) * 128], xk[:],
                                 start=st, stop=sp)
            # evictions (DVE copies round fp32 -> bf16)
            for p in range(2):
                nc.vector.tensor_copy(qT[:, p * T + nt * NB: p * T + (nt + 1) * NB],
                                      psq[p][:])
            nc.vector.tensor_copy(kT[0:64, nt * NB:(nt + 1) * NB], pskv[0:64, :])
            nc.vector.tensor_copy(kT[64:128, nt * NB:(nt + 1) * NB], pskv[0:64, :])
            vTf = vs.tile([64, NB], BF16, tag="vTf")
            nc.vector.tensor_copy(vTf[:], pskv[64:128, :])
            for s in range(4):
                ch = nt * 4 + s
                ptv = ptr.tile([128, HD], BF16, tag="ptv", name=f"ptv{nt}_{s}")
                nc.tensor.transpose(ptv[:], vTf[:, s * 128:(s + 1) * 128],
                                    ident[:])
                nc.vector.tensor_copy(vA[:, ch * 65: ch * 65 + 64], ptv[:])
            if nt % 2 == 1:
                rope_nt(nt)

    # ---------------- phase 2+3: attention + out-projection ---------------
    # two-pass attention per (b, ib, head-pair): (1) scores + exp for all key
    # chunks into an SBUF slab, (2) one dense back-to-back AV matmul stream.
    # Pass 2 of head-pair p overlaps pass 1 of head-pair p+1 on the PE.
    with tc.tile_pool(name="epool", bufs=2) as ep, \
         tc.tile_pool(name="opool", bufs=4) as op, \
         tc.tile_pool(name="pst", bufs=2, space="PSUM") as pst, \
         tc.tile_pool(name="pot", bufs=1, space="PSUM") as pot, \
         tc.tile_pool(name="pmisc", bufs=2, space="PSUM") as pmisc:
        for b in range(B):
            for ib in range(L // NB):
                for p in range(2):
                    qcols = p * T + b * L + ib * NB
                    # one [65,512] accumulator per head of the pair (2 banks)
                    ot = [pot.tile([65, NB], F32, tag=f"ot{o}", name=f"ot{b}_{ib}_{p}_{o}")
                          for o in range(2)]
                    njc = 4 * (ib + 1)
                    eslab = ep.tile([128, 16 * 2 * NB], BF16, tag="eslab",
                                    name=f"eslab{b}_{ib}_{p}")
                    for jc in range(njc):
                        # diagonal chunks: columns < c*128 are fully masked ->
                        # never computed, never read (narrowed matmuls)
                        c = jc - 4 * ib
                        n0 = max(c, 0) * 128
                        # even/odd head score matmuls row-tile at base
                        # partitions 0/64 -> shared [128, 2*NB] PSUM tile
                        # (2 banks) -> one batched exp
                        sc = pst.tile([128, 2 * NB], F32, tag="sc")
                        kcols = slice(b * L + jc * 128, b * L + (jc + 1) * 128)
                        nc.tensor.matmul(sc[:, n0:NB], kT[0:64, kcols],
                                         qT[0:64, qcols + n0:qcols + NB],
                                         start=True, stop=True)
                        nc.tensor.matmul(sc[:, NB + n0:2 * NB], kT[64:128, kcols],
                                         qT[64:128, qcols + n0:qcols + NB],
                                         start=True, stop=True)
                        # exp over the full tile: stale/garbage columns are
                        # finite and never consumed downstream
                        e0 = jc * 2 * NB
                        nc.scalar.activation(eslab[:, e0:e0 + 2 * NB], sc[:],
                                             AF.Exp, scale=float(SCALE))
                        if c >= 0:  # diagonal 128-col band: triangle mask
                            nc.vector.tensor_mul(eslab[:, e0 + n0:e0 + n0 + 128],
                                                 eslab[:, e0 + n0:e0 + n0 + 128], tri[:])
                            nc.vector.tensor_mul(eslab[:, e0 + NB + n0:e0 + NB + n0 + 128],
                                                 eslab[:, e0 + NB + n0:e0 + NB + n0 + 128], tri[:])
                    for jc in range(njc):
                        c = jc - 4 * ib
                        n0 = max(c, 0) * 128
                        e0 = jc * 2 * NB
                        ch = b * 16 + jc  # 16 chunks of 128 tokens per b
                        st, sp = jc == 0, jc == njc - 1
                        nc.tensor.matmul(ot[0][:, n0:NB], vA[:, ch * 65: ch * 65 + 65],
                                         eslab[:, e0 + n0:e0 + NB], start=st, stop=sp)
                        nc.tensor.matmul(ot[1][:, n0:NB], vA[:, ch * 65: ch * 65 + 65],
                                         eslab[:, e0 + NB + n0:e0 + 2 * NB], start=st, stop=sp)
                    for o in range(2):
                        rbase = 64 * o
                        # rounding copy PSUM->SBUF so the f32r matmul is legal
                        drow = tpool.tile([1, NB], F32R, tag="drow")
                        nc.vector.tensor_copy(drow[:], ot[o][64:65, :])
                        bc_ps = pmisc.tile([64, NB], F32, tag="mps", name=f"bc{b}_{ib}_{p}_{o}")
                        nc.tensor.matmul(bc_ps[:], ones2[0:1, 64:128], drow[:],
                                         start=True, stop=True)
                        denb = tpool.tile([64, NB], F32, tag="denb")
                        nc.vector.reciprocal_approx_fast(denb[:], bc_ps[:])
                        nc.vector.tensor_mul(
                            atP[p][rbase:rbase + 64, b * L + ib * NB: b * L + (ib + 1) * NB],
                            ot[o][0:64, :], denb[:])
                # out-projection for this (b, ib) token block
                nt = b * 4 + ib
                for mc in range(16):
                    po = pmisc.tile([128, NB], F32, tag="mps", name=f"po{nt}_{mc}")
                    nc.tensor.matmul(po[:], woT[:, 0 * D + mc * 128: 0 * D + (mc + 1) * 128],
                                     atP[0][:, nt * NB:(nt + 1) * NB],
                                     start=True, stop=False)
                    nc.tensor.matmul(po[:], woT[:, 1 * D + mc * 128: 1 * D + (mc + 1) * 128],
                                     atP[1][:, nt * NB:(nt + 1) * NB],
                                     start=False, stop=True)
                    osb = op.tile([128, NB], BF16, tag="osb")
                    nc.vector.tensor_copy(osb[:], po[:])
                    nc.gpsimd.dma_start(
                        d_out[mc * 128:(mc + 1) * 128, nt * NB:(nt + 1) * NB], osb[:])

    if dump is not None:
        with tc.tile_pool(name="dumpp", bufs=2) as dp:
            for name, t in [("qT", qT), ("kT", kT), ("vA", vA),
                            ("atP0", atP[0]), ("atP1", atP[1])]:
                if name not in dump:
                    continue
                stage = dp.tile(list(t.shape), F32, tag="dstage",
                                name=f"dump_{name}")
                nc.vector.tensor_copy(stage[:], t[:])
                nc.sync.dma_start(dump[name][:], stage[:])


def _deinterleave_rows(w):
    # [H*64, D] -> per-head rows reordered to [even dims | odd dims]
    h = w.shape[0] // HD
    out = np.empty_like(w)
    perm = np.concatenate([np.arange(0, HD, 2), np.arange(1, HD, 2)])
    for i in range(h):
        out[i * HD:(i + 1) * HD] = w[i * HD:(i + 1) * HD][perm]
    return out


def _prep_inputs(x, pos_ids, wq, wk, wv, wo):
    xT = np.ascontiguousarray(x.reshape(T, D).T).astype(ml_dtypes.bfloat16)
    half = HD // 2
    invf = (1.0 / (ROPE_BASE ** (np.arange(half, dtype=np.float32) / half)))
    freqs = pos_ids.astype(np.float32)[None, :] * invf[:, None]  # [32, L]
    cos = np.cos(freqs)
    sin = np.sin(freqs)
    cosb = np.tile(cos, (4, 1))
    sinb = np.tile(sin, (4, 1))
    sinb[0:32] *= -1.0   # even-dim bands get -sin (y_e = x_e*c - x_o*s)
    sinb[64:96] *= -1.0
    # causal triangle for any diagonal 128-col band: query col q attends
    # key row r iff r <= q
    tri = (np.arange(128)[:, None] <= np.arange(128)[None, :])
    onesv = np.zeros((128, 32 * 65), np.float32)
    onesv[:, 64::65] = 1.0
    ones2 = np.concatenate([np.zeros((1, 64), np.float32),
                            np.ones((1, 64), np.float32)], axis=1)
    ones64 = np.ones((1, 64), np.float32)
    ident = np.eye(64, dtype=np.float32)
    in_maps = []
    for c in range(NCORES):
        wq_c = _deinterleave_rows(wq[c * DQ:(c + 1) * DQ])
        wk_c = _deinterleave_rows(wk[c * HD:(c + 1) * HD])
        wv_c = wv[c * HD:(c + 1) * HD]
        wkv_c = np.concatenate([wk_c.T, wv_c.T], axis=1)  # [D, 128]
        wo_c = wo[:, c * DQ:(c + 1) * DQ]
        in_maps.append({
            "xT": xT,
            "wq_t": np.ascontiguousarray(wq_c.T).astype(ml_dtypes.bfloat16),
            "wkv_t": np.ascontiguousarray(wkv_c).astype(ml_dtypes.bfloat16),
            "wo_t": np.ascontiguousarray(wo_c.T).astype(ml_dtypes.bfloat16),
            "cosb": cosb.astype(ml_dtypes.bfloat16),
            "sinb": sinb.astype(ml_dtypes.bfloat16),
            "tri": tri.astype(ml_dtypes.bfloat16),
            "onesv": onesv.astype(ml_dtypes.bfloat16),
            "ones2": ones2,
            "ident": ident.astype(ml_dtypes.bfloat16),
        })
    return in_maps


def kernel(x, pos_ids, wq, wk, wv, wo, _trace=False):
    x = np.asarray(x)
    if "nc" not in _CACHE:
        _CACHE["nc"] = _build_module()
    nc = _CACHE["nc"]
    in_maps = _prep_inputs(np.asarray(x, np.float32), np.asarray(pos_ids),
                           np.asarray(wq, np.float32), np.asarray(wk, np.float32),
                           np.asarray(wv, np.float32), np.asarray(wo, np.float32))
    res = run_bass_kernel_spmd(nc, in_maps, core_ids=list(range(NCORES)),
                               trace=_trace)
    _CACHE["last_results"] = res
    acc = np.zeros((D, T), np.float32)
    for r in res.results:
        acc += r["outT"].astype(np.float32)
    return np.ascontiguousarray(acc.T).reshape(B, L, D)


# revision 31
# speedup vs baseline: 1.0266x; 1.0266x over previous
"""GQA attention (B=2, L=2048, D=2048, Hq=32, Hkv=8, hd=64) on 8 TRN2 cores.

Tensor-parallel over heads: core c owns q heads 4c..4c+3 and kv head c.
Each core computes a partial output (wo input-dim shard); host sums partials.

Per-core layouts (feature-on-partition, "transposed" convention):
  xT      [2048, 4096]   x transposed (shared by all cores), bf16
  wq_t    [2048, 256]    wq shard rows, per-head [even|odd] dim perm, T, bf16
  wk_t    [2048, 64]     wk shard rows, [even|odd] perm, transposed, bf16
  wv_t    [2048, 64]     wv shard rows (natural order), transposed, bf16
  wo_t    [256, 2048]    wo columns shard, transposed, bf16
  cosb/sinb [128, 2048]  host-computed RoPE tables (sign baked into sin), bf16
  tri     [128, 128]     causal triangle mask (query col q attends key row r
                         iff r <= q), bf16
  outT    [2048, 4096]   partial output, transposed, bf16 (host: sum, T)

Kernel phases: QKV projection -> RoPE -> flash-style attention (S.T layout,
no-max softmax via ones-augmented V matmul for the denominator) -> out proj.
bf16 matmul pipeline; even/odd heads of a pair are row-packed (K=64 tiles at
base partitions 0/64) into one [128,1024] PSUM tile -> single batched exp per
(jc, head-pair). Diagonal-block matmuls are narrowed to the causally valid
column range; fully-masked columns are never written or read.
"""
import ml_dtypes
import numpy as np
from contextlib import ExitStack

import concourse.bass as bass
import concourse.mybir as mybir
import concourse.tile as tile
from concourse import bacc
from concourse.bass_utils import run_bass_kernel_spmd

F32 = mybir.dt.float32
F32R = mybir.dt.float32r
BF16 = mybir.dt.bfloat16
I32 = mybir.dt.int32
AF = mybir.ActivationFunctionType
ALU = mybir.AluOpType

B, L, D = 2, 2048, 2048
HQ, HKV, HD = 32, 8, 64
NCORES = 8
HL = HQ // NCORES          # 4 q heads per core
DQ = HL * HD               # 256 local q features
T = B * L                  # 4096 tokens
NB = 512                   # token block
NT = T // NB               # 8 token blocks
KC = D // 128              # 16 contraction chunks
ROPE_BASE = 10000.0
SCALE = 1.0 / np.sqrt(HD)

_CACHE = {}


def _build_module():
    nc = bacc.Bacc("TRN2", target_bir_lowering=False, debug=False,
                   num_devices=NCORES)

    d_xT = nc.dram_tensor("xT", [D, T], BF16, kind="ExternalInput").ap()
    d_wq = nc.dram_tensor("wq_t", [D, DQ], BF16, kind="ExternalInput").ap()
    d_wkv = nc.dram_tensor("wkv_t", [D, 128], BF16, kind="ExternalInput").ap()
    d_wo = nc.dram_tensor("wo_t", [DQ, D], BF16, kind="ExternalInput").ap()
    d_ident = nc.dram_tensor("ident", [64, 64], BF16, kind="ExternalInput").ap()
    d_cos = nc.dram_tensor("cosb", [128, L], BF16, kind="ExternalInput").ap()
    d_sin = nc.dram_tensor("sinb", [128, L], BF16, kind="ExternalInput").ap()
    d_tri = nc.dram_tensor("tri", [128, 128], BF16, kind="ExternalInput").ap()
    d_onesv = nc.dram_tensor("onesv", [128, 32 * 65], BF16, kind="ExternalInput").ap()
    d_ones2 = nc.dram_tensor("ones2", [1, 128], F32R, kind="ExternalInput").ap()
    d_out = nc.dram_tensor("outT", [D, T], BF16, kind="ExternalOutput").ap()

    with tile.TileContext(nc) as tc, ExitStack() as ctx, \
         nc.allow_low_precision(reason="bf16 matmul pipeline"):
        _kernel(tc, ctx, d_xT, d_wq, d_wkv, d_wo, d_cos, d_sin, d_tri,
                d_onesv, d_ones2, d_ident, d_out)

    nc.compile()
    return nc


def _kernel(tc, ctx, d_xT, d_wq, d_wkv, d_wo, d_cos, d_sin, d_tri,
            d_onesv, d_ones2, d_ident, d_out, dump=None):
    nc = tc.nc

    wpool = ctx.enter_context(tc.tile_pool(name="weights", bufs=1))
    spool = ctx.enter_context(tc.tile_pool(name="state", bufs=1))

    # ---------------- persistent SBUF tensors ----------------
    # wq/wkv chunk DMAs are interleaved with the first x-block loads inside
    # the projection loop (same queue -> first matmul starts at ~1.5us).
    # Everything not needed immediately goes on the GpSimd-triggered queue.
    wqT = wpool.tile([128, KC * DQ], BF16, tag="wqT")      # 8KB/part
    wkvT = wpool.tile([128, KC * 128], BF16, tag="wkvT")   # 4KB
    woT = wpool.tile([128, 2 * D], BF16, tag="woT")        # 8KB

    c128 = spool.tile([128, L], BF16, tag="c128")          # 4KB
    s128 = spool.tile([128, L], BF16, tag="s128")          # 4KB
    nc.gpsimd.dma_start(c128[:], d_cos[:])
    nc.gpsimd.dma_start(s128[:], d_sin[:])
    tri = spool.tile([128, 128], BF16, tag="tri")
    nc.gpsimd.dma_start(tri[:], d_tri[:])
    ident = wpool.tile([64, 64], BF16, tag="ident")
    nc.gpsimd.dma_start(ident[:], d_ident[:])
    # ones2 = [0]*64 + [1]*64: slice [64:128] is an all-ones [1,64]
    # stationary; the full [1,128] broadcasts only into rows 64:128
    ones2 = wpool.tile([1, 128], F32R, tag="ones2")
    nc.gpsimd.dma_start(ones2[:], d_ones2[:])

    # v natural layout + interleaved ones column: chunk ch = cols [65ch, 65ch+65)
    vA = spool.tile([128, 32 * 65], BF16, tag="vA")        # 4.2KB
    # ones columns (cols 65ch+64) via one contiguous full-tile DMA; data
    # cols are zero-filled here and overwritten by the v evictions
    nc.gpsimd.dma_start(vA[:], d_onesv[:])
    for kc2 in range(2):
        nc.gpsimd.dma_start(woT[:, kc2 * D:(kc2 + 1) * D],
                            d_wo[kc2 * 128:(kc2 + 1) * 128, :])

    # qT: [128, HL/2 * T]; head pair p on partitions (even head rows 0:64,
    # odd head rows 64:128); within a head [even dims | odd dims].
    qT = spool.tile([128, 2 * T], BF16, tag="qT")          # 16KB
    # kT duplicated on partitions 64:128 so odd-head matmuls row-tile at 64.
    kT = spool.tile([128, T], BF16, tag="kT")              # 8KB
    # attention output, transposed: head pair tiles, b-major columns
    atP = [spool.tile([128, T], BF16, tag=f"atP{p}", name=f"atP{p}") for p in range(2)]  # 16KB

    # ---------------- phase 1.5: RoPE (all-bf16 DVE ops) -------------------
    tpool = ctx.enter_context(tc.tile_pool(name="tmp", bufs=2))

    RW = 2 * NB  # rope over two token blocks at once (amortize DVE op cost)

    def rope(dst, cols, l0):
        # y = x*cos + swap(x)*sgn*sin, swap = exchange 32-row halves per head
        u = tpool.tile([128, RW], BF16, tag="ropeU")
        w = tpool.tile([128, RW], BF16, tag="ropeW")
        xsw = tpool.tile([128, RW], BF16, tag="ropeX")
        for band in range(4):
            r0, r1 = band * 32, (band ^ 1) * 32
            nc.vector.tensor_copy(xsw[r0:r0 + 32, :], dst[r1:r1 + 32, cols])
        nc.vector.tensor_mul(u[:], dst[:, cols], c128[:, l0:l0 + RW])
        nc.vector.tensor_mul(w[:], xsw[:], s128[:, l0:l0 + RW])
        nc.vector.tensor_add(dst[:, cols], u[:], w[:])

    def rope_nt(nt):
        # called on odd nt: ropes token blocks nt-1 and nt together
        b, l0 = nt // 4, (nt % 4 - 1) * NB
        for p in range(2):
            c0 = p * T + b * L + l0
            rope(qT, slice(c0, c0 + RW), l0)
        rope(kT, slice(b * L + l0, b * L + l0 + RW), l0)

    # ---------------- phase 1: QKV projections ----------------------------
    # k and v projections share one [128,128] stationary (cols 0:64 = wk,
    # 64:128 = wv); v comes out feature-major and is PE-transposed into vA.
    with tc.tile_pool(name="xs", bufs=6) as xs, \
         tc.tile_pool(name="vstage", bufs=2) as vs, \
         tc.tile_pool(name="pproj", bufs=2, space="PSUM") as pq, \
         tc.tile_pool(name="pprojkv", bufs=2, space="PSUM") as pkv, \
         tc.tile_pool(name="ptr", bufs=2, space="PSUM") as ptr:
        for nt in range(NT):
            psq = [pq.tile([128, NB], F32, tag=f"psq{i}", name=f"psq{nt}_{i}") for i in range(2)]
            pskv = pkv.tile([128, NB], F32, tag="pskv", name=f"pskv{nt}")
            for kc in range(KC):
                xk = xs.tile([128, NB], BF16, tag="xk")
                if nt == 0:
                    nc.sync.dma_start(wqT[:, kc * DQ:(kc + 1) * DQ],
                                      d_wq[kc * 128:(kc + 1) * 128, :])
                    nc.sync.dma_start(wkvT[:, kc * 128:(kc + 1) * 128],
                                      d_wkv[kc * 128:(kc + 1) * 128, :])
                nc.sync.dma_start(
                    xk[:], d_xT[kc * 128:(kc + 1) * 128, nt * NB:(nt + 1) * NB])
                st, sp = kc == 0, kc == KC - 1
                for p in range(2):
                    nc.tensor.matmul(
                        psq[p][:], wqT[:, kc * DQ + p * 128: kc * DQ + (p + 1) * 128],
                        xk[:], start=st, stop=sp)
                nc.tensor.matmul(pskv[:], wkvT[:, kc * 128:(kc + 1) * 128], xk[:],
                                 start=st, stop=sp)
            # evictions (DVE copies round fp32 -> bf16)
            for p in range(2):
                nc.vector.tensor_copy(qT[:, p * T + nt * NB: p * T + (nt + 1) * NB],
                                      psq[p][:])
            nc.vector.tensor_copy(kT[0:64, nt * NB:(nt + 1) * NB], pskv[0:64, :])
            nc.vector.tensor_copy(kT[64:128, nt * NB:(nt + 1) * NB], pskv[0:64, :])
            vTf = vs.tile([64, NB], BF16, tag="vTf")
            nc.vector.tensor_copy(vTf[:], pskv[64:128, :])
            for s in range(4):
                ch = nt * 4 + s
                ptv = ptr.tile([128, HD], BF16, tag="ptv", name=f"ptv{nt}_{s}")
                nc.tensor.transpose(ptv[:], vTf[:, s * 128:(s + 1) * 128],
                                    ident[:])
                nc.vector.tensor_copy(vA[:, ch * 65: ch * 65 + 64], ptv[:])
            if nt % 2 == 1:
                rope_nt(nt)

    # ---------------- phase 2+3: attention + out-projection ---------------
    # two-pass attention per (b, ib, head-pair): (1) scores + exp for all key
    # chunks into an SBUF slab, (2) one dense back-to-back AV matmul stream.
    # Pass 2 of head-pair p overlaps pass 1 of head-pair p+1 on the PE.
    with tc.tile_pool(name="epool", bufs=2) as ep, \
         tc.tile_pool(name="opool", bufs=6) as op, \
         tc.tile_pool(name="pst", bufs=2, space="PSUM") as pst, \
         tc.tile_pool(name="pot", bufs=1, space="PSUM") as pot, \
         tc.tile_pool(name="pmisc", bufs=2, space="PSUM") as pmisc:
        for b in range(B):
            for ib in range(L // NB):
                for p in range(2):
                    qcols = p * T + b * L + ib * NB
                    # one [65,512] accumulator per head of the pair (2 banks)
                    ot = [pot.tile([65, NB], F32, tag=f"ot{o}", name=f"ot{b}_{ib}_{p}_{o}")
                          for o in range(2)]
                    njc = 4 * (ib + 1)
                    eslab = ep.tile([128, 16 * 2 * NB], BF16, tag="eslab",
                                    name=f"eslab{b}_{ib}_{p}")
                    for jc in range(njc):
                        # diagonal chunks: columns < c*128 are fully masked ->
                        # never computed, never read (narrowed matmuls)
                        c = jc - 4 * ib
                        n0 = max(c, 0) * 128
                        # even/odd head score matmuls row-tile at base
                        # partitions 0/64 -> shared [128, 2*NB] PSUM tile
                        # (2 banks) -> one batched exp
                        sc = pst.tile([128, 2 * NB], F32, tag="sc")
                        kcols = slice(b * L + jc * 128, b * L + (jc + 1) * 128)
                        nc.tensor.matmul(sc[:, n0:NB], kT[0:64, kcols],
                                         qT[0:64, qcols + n0:qcols + NB],
                                         start=True, stop=True)
                        nc.tensor.matmul(sc[:, NB + n0:2 * NB], kT[64:128, kcols],
                                         qT[64:128, qcols + n0:qcols + NB],
                                         start=True, stop=True)
                        # exp over the full tile: stale/garbage columns are
                        # finite and never consumed downstream
                        e0 = jc * 2 * NB
                        nc.scalar.activation(eslab[:, e0:e0 + 2 * NB], sc[:],
                                             AF.Exp, scale=float(SCALE))
                        if c >= 0:  # diagonal 128-col band: triangle mask
                            nc.vector.tensor_mul(eslab[:, e0 + n0:e0 + n0 + 128],
                                                 eslab[:, e0 + n0:e0 + n0 + 128], tri[:])
                            nc.vector.tensor_mul(eslab[:, e0 + NB + n0:e0 + NB + n0 + 128],
                                                 eslab[:, e0 + NB + n0:e0 + NB + n0 + 128], tri[:])
                    for jc in range(njc):
                        c = jc - 4 * ib
                        n0 = max(c, 0) * 128
                        e0 = jc * 2 * NB
                        ch = b * 16 + jc  # 16 chunks of 128 tokens per b
                        st, sp = jc == 0, jc == njc - 1
                        nc.tensor.matmul(ot[0][:, n0:NB], vA[:, ch * 65: ch * 65 + 65],
                                         eslab[:, e0 + n0:e0 + NB], start=st, stop=sp)
                        nc.tensor.matmul(ot[1][:, n0:NB], vA[:, ch * 65: ch * 65 + 65],
                                         eslab[:, e0 + NB + n0:e0 + 2 * NB], start=st, stop=sp)
                    for o in range(2):
                        rbase = 64 * o
                        # rounding copy PSUM->SBUF so the f32r matmul is legal
                        drow = tpool.tile([1, NB], F32R, tag="drow")
                        nc.vector.tensor_copy(drow[:], ot[o][64:65, :])
                        # stage ot out of PSUM right away so the bank frees
                        # for the next head-pair's AV accumulation
                        ot_sb = tpool.tile([64, NB], F32, tag="ot_sb")
                        nc.vector.tensor_copy(ot_sb[:], ot[o][0:64, :])
                        bc_ps = pmisc.tile([64, NB], F32, tag="mps", name=f"bc{b}_{ib}_{p}_{o}")
                        nc.tensor.matmul(bc_ps[:], ones2[0:1, 64:128], drow[:],
                                         start=True, stop=True)
                        denb = tpool.tile([64, NB], F32, tag="denb")
                        nc.vector.reciprocal_approx_fast(denb[:], bc_ps[:])
                        nc.vector.tensor_mul(
                            atP[p][rbase:rbase + 64, b * L + ib * NB: b * L + (ib + 1) * NB],
                            ot_sb[:], denb[:])
                # out-projection for this (b, ib) token block
                nt = b * 4 + ib
                for mc in range(16):
                    po = pmisc.tile([128, NB], F32, tag="mps", name=f"po{nt}_{mc}")
                    nc.tensor.matmul(po[:], woT[:, 0 * D + mc * 128: 0 * D + (mc + 1) * 128],
                                     atP[0][:, nt * NB:(nt + 1) * NB],
                                     start=True, stop=False)
                    nc.tensor.matmul(po[:], woT[:, 1 * D + mc * 128: 1 * D + (mc + 1) * 128],
                                     atP[1][:, nt * NB:(nt + 1) * NB],
                                     start=False, stop=True)
                    osb = op.tile([128, NB], BF16, tag="osb")
                    nc.vector.tensor_copy(osb[:], po[:])
                    nc.gpsimd.dma_start(
                        d_out[mc * 128:(mc + 1) * 128, nt * NB:(nt + 1) * NB], osb[:])

    if dump is not None:
        with tc.tile_pool(name="dumpp", bufs=2) as dp:
            for name, t in [("qT", qT), ("kT", kT), ("vA", vA),
                            ("atP0", atP[0]), ("atP1", atP[1])]:
                if name not in dump:
                    continue
                stage = dp.tile(list(t.shape), F32, tag="dstage",
                                name=f"dump_{name}")
                nc.vector.tensor_copy(stage[:], t[:])
                nc.sync.dma_start(dump[name][:], stage[:])


def _deinterleave_rows(w):
    # [H*64, D] -> per-head rows reordered to [even dims | odd dims]
    h = w.shape[0] // HD
    out = np.empty_like(w)
    perm = np.concatenate([np.arange(0, HD, 2), np.arange(1, HD, 2)])
    for i in range(h):
        out[i * HD:(i + 1) * HD] = w[i * HD:(i + 1) * HD][perm]
    return out


def _prep_inputs(x, pos_ids, wq, wk, wv, wo):
    xT = np.ascontiguousarray(x.reshape(T, D).T).astype(ml_dtypes.bfloat16)
    half = HD // 2
    invf = (1.0 / (ROPE_BASE ** (np.arange(half, dtype=np.float32) / half)))
    freqs = pos_ids.astype(np.float32)[None, :] * invf[:, None]  # [32, L]
    cos = np.cos(freqs)
    sin = np.sin(freqs)
    cosb = np.tile(cos, (4, 1))
    sinb = np.tile(sin, (4, 1))
    sinb[0:32] *= -1.0   # even-dim bands get -sin (y_e = x_e*c - x_o*s)
    sinb[64:96] *= -1.0
    # causal triangle for any diagonal 128-col band: query col q attends
    # key row r iff r <= q
    tri = (np.arange(128)[:, None] <= np.arange(128)[None, :])
    onesv = np.zeros((128, 32 * 65), np.float32)
    onesv[:, 64::65] = 1.0
    ones2 = np.concatenate([np.zeros((1, 64), np.float32),
                            np.ones((1, 64), np.float32)], axis=1)
    ones64 = np.ones((1, 64), np.float32)
    ident = np.eye(64, dtype=np.float32)
    in_maps = []
    for c in range(NCORES):
        wq_c = _deinterleave_rows(wq[c * DQ:(c + 1) * DQ])
        wk_c = _deinterleave_rows(wk[c * HD:(c + 1) * HD])
        wv_c = wv[c * HD:(c + 1) * HD]
        wkv_c = np.concatenate([wk_c.T, wv_c.T], axis=1)  # [D, 128]
        wo_c = wo[:, c * DQ:(c + 1) * DQ]
        in_maps.append({
            "xT": xT,
            "wq_t": np.ascontiguousarray(wq_c.T).astype(ml_dtypes.bfloat16),
            "wkv_t": np.ascontiguousarray(wkv_c).astype(ml_dtypes.bfloat16),
            "wo_t": np.ascontiguousarray(wo_c.T).astype(ml_dtypes.bfloat16),
            "cosb": cosb.astype(ml_dtypes.bfloat16),
            "sinb": sinb.astype(ml_dtypes.bfloat16),
            "tri": tri.astype(ml_dtypes.bfloat16),
            "onesv": onesv.astype(ml_dtypes.bfloat16),
            "ones2": ones2,
            "ident": ident.astype(ml_dtypes.bfloat16),
        })
    return in_maps


def kernel(x, pos_ids, wq, wk, wv, wo, _trace=False):
    x = np.asarray(x)
    if "nc" not in _CACHE:
        _CACHE["nc"] = _build_module()
    nc = _CACHE["nc"]
    in_maps = _prep_inputs(np.asarray(x, np.float32), np.asarray(pos_ids),
                           np.asarray(wq, np.float32), np.asarray(wk, np.float32),
                           np.asarray(wv, np.float32), np.asarray(wo, np.float32))
    res = run_bass_kernel_spmd(nc, in_maps, core_ids=list(range(NCORES)),
                               trace=_trace)
    _CACHE["last_results"] = res
    acc = np.zeros((D, T), np.float32)
    for r in res.results:
        acc += r["outT"].astype(np.float32)
    return np.ascontiguousarray(acc.T).reshape(B, L, D)


# revision 32
# speedup vs baseline: 1.0287x; 1.0021x over previous
"""GQA attention (B=2, L=2048, D=2048, Hq=32, Hkv=8, hd=64) on 8 TRN2 cores.

Tensor-parallel over heads: core c owns q heads 4c..4c+3 and kv head c.
Each core computes a partial output (wo input-dim shard); host sums partials.

Per-core layouts (feature-on-partition, "transposed" convention):
  xT      [2048, 4096]   x transposed (shared by all cores), bf16
  wq_t    [2048, 256]    wq shard rows, per-head [even|odd] dim perm, T, bf16
  wk_t    [2048, 64]     wk shard rows, [even|odd] perm, transposed, bf16
  wv_t    [2048, 64]     wv shard rows (natural order), transposed, bf16
  wo_t    [256, 2048]    wo columns shard, transposed, bf16
  cosb/sinb [128, 2048]  host-computed RoPE tables (sign baked into sin), bf16
  tri     [128, 128]     causal triangle mask (query col q attends key row r
                         iff r <= q), bf16
  outT    [2048, 4096]   partial output, transposed, bf16 (host: sum, T)

Kernel phases: QKV projection -> RoPE -> flash-style attention (S.T layout,
no-max softmax via ones-augmented V matmul for the denominator) -> out proj.
bf16 matmul pipeline; even/odd heads of a pair are row-packed (K=64 tiles at
base partitions 0/64) into one [128,1024] PSUM tile -> single batched exp per
(jc, head-pair). Diagonal-block matmuls are narrowed to the causally valid
column range; fully-masked columns are never written or read.
"""
import ml_dtypes
import numpy as np
from contextlib import ExitStack

import concourse.bass as bass
import concourse.mybir as mybir
import concourse.tile as tile
from concourse import bacc
from concourse.bass_utils import run_bass_kernel_spmd

F32 = mybir.dt.float32
F32R = mybir.dt.float32r
BF16 = mybir.dt.bfloat16
I32 = mybir.dt.int32
AF = mybir.ActivationFunctionType
ALU = mybir.AluOpType

B, L, D = 2, 2048, 2048
HQ, HKV, HD = 32, 8, 64
NCORES = 8
HL = HQ // NCORES          # 4 q heads per core
DQ = HL * HD               # 256 local q features
T = B * L                  # 4096 tokens
NB = 512                   # token block
NT = T // NB               # 8 token blocks
KC = D // 128              # 16 contraction chunks
ROPE_BASE = 10000.0
SCALE = 1.0 / np.sqrt(HD)

_CACHE = {}


def _build_module():
    nc = bacc.Bacc("TRN2", target_bir_lowering=False, debug=False,
                   num_devices=NCORES)

    d_xT = nc.dram_tensor("xT", [D, T], BF16, kind="ExternalInput").ap()
    d_wq = nc.dram_tensor("wq_t", [D, DQ], BF16, kind="ExternalInput").ap()
    d_wkv = nc.dram_tensor("wkv_t", [D, 128], BF16, kind="ExternalInput").ap()
    d_wo = nc.dram_tensor("wo_t", [DQ, D], BF16, kind="ExternalInput").ap()
    d_ident = nc.dram_tensor("ident", [64, 64], BF16, kind="ExternalInput").ap()
    d_cos = nc.dram_tensor("cosb", [128, L], BF16, kind="ExternalInput").ap()
    d_sin = nc.dram_tensor("sinb", [128, L], BF16, kind="ExternalInput").ap()
    d_tri = nc.dram_tensor("tri", [128, 128], BF16, kind="ExternalInput").ap()
    d_onesv = nc.dram_tensor("onesv", [128, 32 * 65], BF16, kind="ExternalInput").ap()
    d_ones2 = nc.dram_tensor("ones2", [1, 128], F32R, kind="ExternalInput").ap()
    d_out = nc.dram_tensor("outT", [D, T], BF16, kind="ExternalOutput").ap()

    with tile.TileContext(nc) as tc, ExitStack() as ctx, \
         nc.allow_low_precision(reason="bf16 matmul pipeline"):
        _kernel(tc, ctx, d_xT, d_wq, d_wkv, d_wo, d_cos, d_sin, d_tri,
                d_onesv, d_ones2, d_ident, d_out)

    nc.compile()
    return nc


def _kernel(tc, ctx, d_xT, d_wq, d_wkv, d_wo, d_cos, d_sin, d_tri,
            d_onesv, d_ones2, d_ident, d_out, dump=None):
    nc = tc.nc

    wpool = ctx.enter_context(tc.tile_pool(name="weights", bufs=1))
    spool = ctx.enter_context(tc.tile_pool(name="state", bufs=1))

    # ---------------- persistent SBUF tensors ----------------
    # wq/wkv chunk DMAs are interleaved with the first x-block loads inside
    # the projection loop (same queue -> first matmul starts at ~1.5us).
    # Everything not needed immediately goes on the GpSimd-triggered queue.
    wqT = wpool.tile([128, KC * DQ], BF16, tag="wqT")      # 8KB/part
    wkvT = wpool.tile([128, KC * 128], BF16, tag="wkvT")   # 4KB
    woT = wpool.tile([128, 2 * D], BF16, tag="woT")        # 8KB

    c128 = spool.tile([128, L], BF16, tag="c128")          # 4KB
    s128 = spool.tile([128, L], BF16, tag="s128")          # 4KB
    nc.gpsimd.dma_start(c128[:], d_cos[:])
    nc.gpsimd.dma_start(s128[:], d_sin[:])
    tri = spool.tile([128, 128], BF16, tag="tri")
    nc.gpsimd.dma_start(tri[:], d_tri[:])
    ident = wpool.tile([64, 64], BF16, tag="ident")
    nc.gpsimd.dma_start(ident[:], d_ident[:])
    # ones2 = [0]*64 + [1]*64: slice [64:128] is an all-ones [1,64]
    # stationary; the full [1,128] broadcasts only into rows 64:128
    ones2 = wpool.tile([1, 128], F32R, tag="ones2")
    nc.gpsimd.dma_start(ones2[:], d_ones2[:])

    # v natural layout + interleaved ones column: chunk ch = cols [65ch, 65ch+65)
    vA = spool.tile([128, 32 * 65], BF16, tag="vA")        # 4.2KB
    # ones columns (cols 65ch+64) via one contiguous full-tile DMA; data
    # cols are zero-filled here and overwritten by the v evictions
    nc.gpsimd.dma_start(vA[:], d_onesv[:])
    for kc2 in range(2):
        nc.gpsimd.dma_start(woT[:, kc2 * D:(kc2 + 1) * D],
                            d_wo[kc2 * 128:(kc2 + 1) * 128, :])

    # qT: [128, HL/2 * T]; head pair p on partitions (even head rows 0:64,
    # odd head rows 64:128); within a head [even dims | odd dims].
    qT = spool.tile([128, 2 * T], BF16, tag="qT")          # 16KB
    # kT duplicated on partitions 64:128 so odd-head matmuls row-tile at 64.
    kT = spool.tile([128, T], BF16, tag="kT")              # 8KB
    # attention output, transposed: head pair tiles, b-major columns
    atP = [spool.tile([128, T], BF16, tag=f"atP{p}", name=f"atP{p}") for p in range(2)]  # 16KB

    # ---------------- phase 1.5: RoPE (all-bf16 DVE ops) -------------------
    tpool = ctx.enter_context(tc.tile_pool(name="tmp", bufs=2))

    RW = 2 * NB  # rope over two token blocks at once (amortize DVE op cost)

    def rope(dst, cols, l0):
        # y = x*cos + swap(x)*sgn*sin, swap = exchange 32-row halves per head
        u = tpool.tile([128, RW], BF16, tag="ropeU")
        w = tpool.tile([128, RW], BF16, tag="ropeW")
        xsw = tpool.tile([128, RW], BF16, tag="ropeX")
        for band in range(4):
            r0, r1 = band * 32, (band ^ 1) * 32
            nc.vector.tensor_copy(xsw[r0:r0 + 32, :], dst[r1:r1 + 32, cols])
        nc.vector.tensor_mul(u[:], dst[:, cols], c128[:, l0:l0 + RW])
        nc.vector.tensor_mul(w[:], xsw[:], s128[:, l0:l0 + RW])
        nc.vector.tensor_add(dst[:, cols], u[:], w[:])

    def rope_nt(nt):
        # called on odd nt: ropes token blocks nt-1 and nt together
        b, l0 = nt // 4, (nt % 4 - 1) * NB
        for p in range(2):
            c0 = p * T + b * L + l0
            rope(qT, slice(c0, c0 + RW), l0)
        rope(kT, slice(b * L + l0, b * L + l0 + RW), l0)

    # ---------------- phase 1: QKV projections ----------------------------
    # k and v projections share one [128,128] stationary (cols 0:64 = wk,
    # 64:128 = wv); v comes out feature-major and is PE-transposed into vA.
    with tc.tile_pool(name="xs", bufs=10) as xs, \
         tc.tile_pool(name="vstage", bufs=2) as vs, \
         tc.tile_pool(name="pproj", bufs=2, space="PSUM") as pq, \
         tc.tile_pool(name="pprojkv", bufs=2, space="PSUM") as pkv, \
         tc.tile_pool(name="ptr", bufs=2, space="PSUM") as ptr:
        for nt in range(NT):
            psq = [pq.tile([128, NB], F32, tag=f"psq{i}", name=f"psq{nt}_{i}") for i in range(2)]
            pskv = pkv.tile([128, NB], F32, tag="pskv", name=f"pskv{nt}")
            for kc in range(KC):
                xk = xs.tile([128, NB], BF16, tag="xk")
                if nt == 0:
                    nc.sync.dma_start(wqT[:, kc * DQ:(kc + 1) * DQ],
                                      d_wq[kc * 128:(kc + 1) * 128, :])
                    nc.sync.dma_start(wkvT[:, kc * 128:(kc + 1) * 128],
                                      d_wkv[kc * 128:(kc + 1) * 128, :])
                nc.sync.dma_start(
                    xk[:], d_xT[kc * 128:(kc + 1) * 128, nt * NB:(nt + 1) * NB])
                st, sp = kc == 0, kc == KC - 1
                for p in range(2):
                    nc.tensor.matmul(
                        psq[p][:], wqT[:, kc * DQ + p * 128: kc * DQ + (p + 1) * 128],
                        xk[:], start=st, stop=sp)
                nc.tensor.matmul(pskv[:], wkvT[:, kc * 128:(kc + 1) * 128], xk[:],
                                 start=st, stop=sp)
            # evictions (DVE copies round fp32 -> bf16)
            for p in range(2):
                nc.vector.tensor_copy(qT[:, p * T + nt * NB: p * T + (nt + 1) * NB],
                                      psq[p][:])
            nc.vector.tensor_copy(kT[0:64, nt * NB:(nt + 1) * NB], pskv[0:64, :])
            nc.vector.tensor_copy(kT[64:128, nt * NB:(nt + 1) * NB], pskv[0:64, :])
            vTf = vs.tile([64, NB], BF16, tag="vTf")
            nc.vector.tensor_copy(vTf[:], pskv[64:128, :])
            for s in range(4):
                ch = nt * 4 + s
                ptv = ptr.tile([128, HD], BF16, tag="ptv", name=f"ptv{nt}_{s}")
                nc.tensor.transpose(ptv[:], vTf[:, s * 128:(s + 1) * 128],
                                    ident[:])
                nc.vector.tensor_copy(vA[:, ch * 65: ch * 65 + 64], ptv[:])
            if nt % 2 == 1:
                rope_nt(nt)

    # ---------------- phase 2+3: attention + out-projection ---------------
    # two-pass attention per (b, ib, head-pair): (1) scores + exp for all key
    # chunks into an SBUF slab, (2) one dense back-to-back AV matmul stream.
    # Pass 2 of head-pair p overlaps pass 1 of head-pair p+1 on the PE.
    with tc.tile_pool(name="epool", bufs=2) as ep, \
         tc.tile_pool(name="opool", bufs=6) as op, \
         tc.tile_pool(name="pst", bufs=2, space="PSUM") as pst, \
         tc.tile_pool(name="pot", bufs=1, space="PSUM") as pot, \
         tc.tile_pool(name="pmisc", bufs=2, space="PSUM") as pmisc:
        for b in range(B):
            for ib in range(L // NB):
                for p in range(2):
                    qcols = p * T + b * L + ib * NB
                    # one [65,512] accumulator per head of the pair (2 banks)
                    ot = [pot.tile([65, NB], F32, tag=f"ot{o}", name=f"ot{b}_{ib}_{p}_{o}")
                          for o in range(2)]
                    njc = 4 * (ib + 1)
                    eslab = ep.tile([128, 16 * 2 * NB], BF16, tag="eslab",
                                    name=f"eslab{b}_{ib}_{p}")
                    for jc in range(njc):
                        # diagonal chunks: columns < c*128 are fully masked ->
                        # never computed, never read (narrowed matmuls)
                        c = jc - 4 * ib
                        n0 = max(c, 0) * 128
                        # even/odd head score matmuls row-tile at base
                        # partitions 0/64 -> shared [128, 2*NB] PSUM tile
                        # (2 banks) -> one batched exp
                        sc = pst.tile([128, 2 * NB], F32, tag="sc")
                        kcols = slice(b * L + jc * 128, b * L + (jc + 1) * 128)
                        nc.tensor.matmul(sc[:, n0:NB], kT[0:64, kcols],
                                         qT[0:64, qcols + n0:qcols + NB],
                                         start=True, stop=True)
                        nc.tensor.matmul(sc[:, NB + n0:2 * NB], kT[64:128, kcols],
                                         qT[64:128, qcols + n0:qcols + NB],
                                         start=True, stop=True)
                        # exp over the full tile: stale/garbage columns are
                        # finite and never consumed downstream
                        e0 = jc * 2 * NB
                        nc.scalar.activation(eslab[:, e0:e0 + 2 * NB], sc[:],
                                             AF.Exp, scale=float(SCALE))
                        if c >= 0:  # diagonal 128-col band: triangle mask
                            nc.vector.tensor_mul(eslab[:, e0 + n0:e0 + n0 + 128],
                                                 eslab[:, e0 + n0:e0 + n0 + 128], tri[:])
                            nc.vector.tensor_mul(eslab[:, e0 + NB + n0:e0 + NB + n0 + 128],
                                                 eslab[:, e0 + NB + n0:e0 + NB + n0 + 128], tri[:])
                    for jc in range(njc):
                        c = jc - 4 * ib
                        n0 = max(c, 0) * 128
                        e0 = jc * 2 * NB
                        ch = b * 16 + jc  # 16 chunks of 128 tokens per b
                        st, sp = jc == 0, jc == njc - 1
                        nc.tensor.matmul(ot[0][:, n0:NB], vA[:, ch * 65: ch * 65 + 65],
                                         eslab[:, e0 + n0:e0 + NB], start=st, stop=sp)
                        nc.tensor.matmul(ot[1][:, n0:NB], vA[:, ch * 65: ch * 65 + 65],
                                         eslab[:, e0 + NB + n0:e0 + 2 * NB], start=st, stop=sp)
                    for o in range(2):
                        rbase = 64 * o
                        # rounding copy PSUM->SBUF so the f32r matmul is legal
                        drow = tpool.tile([1, NB], F32R, tag="drow")
                        nc.vector.tensor_copy(drow[:], ot[o][64:65, :])
                        # stage ot out of PSUM right away so the bank frees
                        # for the next head-pair's AV accumulation
                        ot_sb = tpool.tile([64, NB], F32, tag="ot_sb")
                        nc.vector.tensor_copy(ot_sb[:], ot[o][0:64, :])
                        bc_ps = pmisc.tile([64, NB], F32, tag="mps", name=f"bc{b}_{ib}_{p}_{o}")
                        nc.tensor.matmul(bc_ps[:], ones2[0:1, 64:128], drow[:],
                                         start=True, stop=True)
                        denb = tpool.tile([64, NB], F32, tag="denb")
                        nc.vector.reciprocal_approx_fast(denb[:], bc_ps[:])
                        nc.vector.tensor_mul(
                            atP[p][rbase:rbase + 64, b * L + ib * NB: b * L + (ib + 1) * NB],
                            ot_sb[:], denb[:])
                # out-projection for this (b, ib) token block
                nt = b * 4 + ib
                for mc in range(16):
                    po = pmisc.tile([128, NB], F32, tag="mps", name=f"po{nt}_{mc}")
                    nc.tensor.matmul(po[:], woT[:, 0 * D + mc * 128: 0 * D + (mc + 1) * 128],
                                     atP[0][:, nt * NB:(nt + 1) * NB],
                                     start=True, stop=False)
                    nc.tensor.matmul(po[:], woT[:, 1 * D + mc * 128: 1 * D + (mc + 1) * 128],
                                     atP[1][:, nt * NB:(nt + 1) * NB],
                                     start=False, stop=True)
                    osb = op.tile([128, NB], BF16, tag="osb")
                    nc.vector.tensor_copy(osb[:], po[:])
                    nc.gpsimd.dma_start(
                        d_out[mc * 128:(mc + 1) * 128, nt * NB:(nt + 1) * NB], osb[:])

    if dump is not None:
        with tc.tile_pool(name="dumpp", bufs=2) as dp:
            for name, t in [("qT", qT), ("kT", kT), ("vA", vA),
                            ("atP0", atP[0]), ("atP1", atP[1])]:
                if name not in dump:
                    continue
                stage = dp.tile(list(t.shape), F32, tag="dstage",
                                name=f"dump_{name}")
                nc.vector.tensor_copy(stage[:], t[:])
                nc.sync.dma_start(dump[name][:], stage[:])


def _deinterleave_rows(w):
    # [H*64, D] -> per-head rows reordered to [even dims | odd dims]
    h = w.shape[0] // HD
    out = np.empty_like(w)
    perm = np.concatenate([np.arange(0, HD, 2), np.arange(1, HD, 2)])
    for i in range(h):
        out[i * HD:(i + 1) * HD] = w[i * HD:(i + 1) * HD][perm]
    return out


def _prep_inputs(x, pos_ids, wq, wk, wv, wo):
    xT = np.ascontiguousarray(x.reshape(T, D).T).astype(ml_dtypes.bfloat16)
    half = HD // 2
    invf = (1.0 / (ROPE_BASE ** (np.arange(half, dtype=np.float32) / half)))
    freqs = pos_ids.astype(np.float32)[None, :] * invf[:, None]  # [32, L]
    cos = np.cos(freqs)
    sin = np.sin(freqs)
    cosb = np.tile(cos, (4, 1))
    sinb = np.tile(sin, (4, 1))
    sinb[0:32] *= -1.0   # even-dim bands get -sin (y_e = x_e*c - x_o*s)
    sinb[64:96] *= -1.0
    # causal triangle for any diagonal 128-col band: query col q attends
    # key row r iff r <= q
    tri = (np.arange(128)[:, None] <= np.arange(128)[None, :])
    onesv = np.zeros((128, 32 * 65), np.float32)
    onesv[:, 64::65] = 1.0
    ones2 = np.concatenate([np.zeros((1, 64), np.float32),
                            np.ones((1, 64), np.float32)], axis=1)
    ones64 = np.ones((1, 64), np.float32)
    ident = np.eye(64, dtype=np.float32)
    in_maps = []
    for c in range(NCORES):
        wq_c = _deinterleave_rows(wq[c * DQ:(c + 1) * DQ])
        wk_c = _deinterleave_rows(wk[c * HD:(c + 1) * HD])
        wv_c = wv[c * HD:(c + 1) * HD]
        wkv_c = np.concatenate([wk_c.T, wv_c.T], axis=1)  # [D, 128]
        wo_c = wo[:, c * DQ:(c + 1) * DQ]
        in_maps.append({
            "xT": xT,
            "wq_t": np.ascontiguousarray(wq_c.T).astype(ml_dtypes.bfloat16),
            "wkv_t": np.ascontiguousarray(wkv_c).astype(ml_dtypes.bfloat16),
            "wo_t": np.ascontiguousarray(wo_c.T).astype(ml_dtypes.bfloat16),
            "cosb": cosb.astype(ml_dtypes.bfloat16),
            "sinb": sinb.astype(ml_dtypes.bfloat16),
            "tri": tri.astype(ml_dtypes.bfloat16),
            "onesv": onesv.astype(ml_dtypes.bfloat16),
            "ones2": ones2,
            "ident": ident.astype(ml_dtypes.bfloat16),
        })
    return in_maps


def kernel(x, pos_ids, wq, wk, wv, wo, _trace=False):
    x = np.asarray(x)
    if "nc" not in _CACHE:
        _CACHE["nc"] = _build_module()
    nc = _CACHE["nc"]
    in_maps = _prep_inputs(np.asarray(x, np.float32), np.asarray(pos_ids),
                           np.asarray(wq, np.float32), np.asarray(wk, np.float32),
                           np.asarray(wv, np.float32), np.asarray(wo, np.float32))
    res = run_bass_kernel_spmd(nc, in_maps, core_ids=list(range(NCORES)),
                               trace=_trace)
    _CACHE["last_results"] = res
    acc = np.zeros((D, T), np.float32)
    for r in res.results:
        acc += r["outT"].astype(np.float32)
    return np.ascontiguousarray(acc.T).reshape(B, L, D)


# revision 34
# speedup vs baseline: 1.0631x; 1.0334x over previous
"""GQA attention (B=2, L=2048, D=2048, Hq=32, Hkv=8, hd=64) on 8 TRN2 cores.

Tensor-parallel over heads: core c owns q heads 4c..4c+3 and kv head c.
Each core computes a partial output (wo input-dim shard); host sums partials.

Per-core layouts (feature-on-partition, "transposed" convention):
  xT      [2048, 4096]   x transposed (shared by all cores), bf16
  wq_t    [2048, 256]    wq shard rows, per-head [even|odd] dim perm, T, bf16
  wk_t    [2048, 64]     wk shard rows, [even|odd] perm, transposed, bf16
  wv_t    [2048, 64]     wv shard rows (natural order), transposed, bf16
  wo_t    [256, 2048]    wo columns shard, transposed, bf16
  cosb/sinb [128, 2048]  host-computed RoPE tables (sign baked into sin), bf16
  tri     [128, 128]     causal triangle mask (query col q attends key row r
                         iff r <= q), bf16
  outT    [2048, 4096]   partial output, transposed, bf16 (host: sum, T)

Kernel phases: QKV projection -> RoPE -> flash-style attention (S.T layout,
no-max softmax via ones-augmented V matmul for the denominator) -> out proj.
bf16 matmul pipeline; even/odd heads of a pair are row-packed (K=64 tiles at
base partitions 0/64) into one [128,1024] PSUM tile -> single batched exp per
(jc, head-pair). Diagonal-block matmuls are narrowed to the causally valid
column range; fully-masked columns are never written or read.
"""
import ml_dtypes
import numpy as np
from contextlib import ExitStack

import concourse.bass as bass
import concourse.mybir as mybir
import concourse.tile as tile
from concourse import bacc
from concourse.bass_utils import run_bass_kernel_spmd

F32 = mybir.dt.float32
F32R = mybir.dt.float32r
BF16 = mybir.dt.bfloat16
I32 = mybir.dt.int32
AF = mybir.ActivationFunctionType
ALU = mybir.AluOpType

B, L, D = 2, 2048, 2048
HQ, HKV, HD = 32, 8, 64
NCORES = 8
HL = HQ // NCORES          # 4 q heads per core
DQ = HL * HD               # 256 local q features
T = B * L                  # 4096 tokens
NB = 512                   # token block
NT = T // NB               # 8 token blocks
KC = D // 128              # 16 contraction chunks
ROPE_BASE = 10000.0
SCALE = 1.0 / np.sqrt(HD)

_CACHE = {}


def _build_module():
    nc = bacc.Bacc("TRN2", target_bir_lowering=False, debug=False,
                   num_devices=NCORES)

    d_xT = nc.dram_tensor("xT", [D, T], BF16, kind="ExternalInput").ap()
    d_wq = nc.dram_tensor("wq_t", [D, DQ], BF16, kind="ExternalInput").ap()
    d_wkv = nc.dram_tensor("wkv_t", [D, 128], BF16, kind="ExternalInput").ap()
    d_wo = nc.dram_tensor("wo_t", [DQ, D], BF16, kind="ExternalInput").ap()
    d_ident = nc.dram_tensor("ident", [64, 64], BF16, kind="ExternalInput").ap()
    d_cos = nc.dram_tensor("cosb", [128, L], BF16, kind="ExternalInput").ap()
    d_sin = nc.dram_tensor("sinb", [128, L], BF16, kind="ExternalInput").ap()
    d_tri = nc.dram_tensor("tri", [128, 128], BF16, kind="ExternalInput").ap()
    d_onesv = nc.dram_tensor("onesv", [128, 32 * 65], BF16, kind="ExternalInput").ap()
    d_ones2 = nc.dram_tensor("ones2", [1, 128], F32R, kind="ExternalInput").ap()
    d_out = nc.dram_tensor("outT", [D, T], BF16, kind="ExternalOutput").ap()

    with tile.TileContext(nc) as tc, ExitStack() as ctx, \
         nc.allow_low_precision(reason="bf16 matmul pipeline"):
        _kernel(tc, ctx, d_xT, d_wq, d_wkv, d_wo, d_cos, d_sin, d_tri,
                d_onesv, d_ones2, d_ident, d_out)

    nc.compile()
    return nc


def _kernel(tc, ctx, d_xT, d_wq, d_wkv, d_wo, d_cos, d_sin, d_tri,
            d_onesv, d_ones2, d_ident, d_out, dump=None):
    nc = tc.nc

    wpool = ctx.enter_context(tc.tile_pool(name="weights", bufs=1))
    spool = ctx.enter_context(tc.tile_pool(name="state", bufs=1))

    # ---------------- persistent SBUF tensors ----------------
    # wq/wkv chunk DMAs are interleaved with the first x-block loads inside
    # the projection loop (same queue -> first matmul starts at ~1.5us).
    # Everything not needed immediately goes on the GpSimd-triggered queue.
    wqT = wpool.tile([128, KC * DQ], BF16, tag="wqT")      # 8KB/part
    wkvT = wpool.tile([128, KC * 128], BF16, tag="wkvT")   # 4KB
    woT = wpool.tile([128, 2 * D], BF16, tag="woT")        # 8KB
    for kc in range(KC):
        nc.gpsimd.dma_start(wqT[:, kc * DQ:(kc + 1) * DQ],
                            d_wq[kc * 128:(kc + 1) * 128, :])
        nc.gpsimd.dma_start(wkvT[:, kc * 128:(kc + 1) * 128],
                            d_wkv[kc * 128:(kc + 1) * 128, :])

    c128 = spool.tile([128, L], BF16, tag="c128")          # 4KB
    s128 = spool.tile([128, L], BF16, tag="s128")          # 4KB
    nc.gpsimd.dma_start(c128[:], d_cos[:])
    nc.gpsimd.dma_start(s128[:], d_sin[:])
    tri = spool.tile([128, 128], BF16, tag="tri")
    nc.gpsimd.dma_start(tri[:], d_tri[:])
    ident = wpool.tile([64, 64], BF16, tag="ident")
    nc.gpsimd.dma_start(ident[:], d_ident[:])
    # ones2 = [0]*64 + [1]*64: slice [64:128] is an all-ones [1,64]
    # stationary; the full [1,128] broadcasts only into rows 64:128
    ones2 = wpool.tile([1, 128], F32R, tag="ones2")
    nc.gpsimd.dma_start(ones2[:], d_ones2[:])

    # v natural layout + interleaved ones column: chunk ch = cols [65ch, 65ch+65)
    vA = spool.tile([128, 32 * 65], BF16, tag="vA")        # 4.2KB
    # ones columns (cols 65ch+64) via one contiguous full-tile DMA; data
    # cols are zero-filled here and overwritten by the v evictions
    nc.gpsimd.dma_start(vA[:], d_onesv[:])
    for kc2 in range(2):
        nc.gpsimd.dma_start(woT[:, kc2 * D:(kc2 + 1) * D],
                            d_wo[kc2 * 128:(kc2 + 1) * 128, :])

    # qT: [128, HL/2 * T]; head pair p on partitions (even head rows 0:64,
    # odd head rows 64:128); within a head [even dims | odd dims].
    qT = spool.tile([128, 2 * T], BF16, tag="qT")          # 16KB
    # kT duplicated on partitions 64:128 so odd-head matmuls row-tile at 64.
    kT = spool.tile([128, T], BF16, tag="kT")              # 8KB
    # attention output, transposed: head pair tiles, b-major columns
    atP = [spool.tile([128, T], BF16, tag=f"atP{p}", name=f"atP{p}") for p in range(2)]  # 16KB

    # ---------------- phase 1.5: RoPE (all-bf16 DVE ops) -------------------
    tpool = ctx.enter_context(tc.tile_pool(name="tmp", bufs=2))

    RW = 2 * NB  # rope over two token blocks at once (amortize DVE op cost)

    def rope(dst, cols, l0):
        # y = x*cos + swap(x)*sgn*sin, swap = exchange 32-row halves per head
        u = tpool.tile([128, RW], BF16, tag="ropeU")
        w = tpool.tile([128, RW], BF16, tag="ropeW")
        xsw = tpool.tile([128, RW], BF16, tag="ropeX")
        for band in range(4):
            r0, r1 = band * 32, (band ^ 1) * 32
            nc.vector.tensor_copy(xsw[r0:r0 + 32, :], dst[r1:r1 + 32, cols])
        nc.vector.tensor_mul(u[:], dst[:, cols], c128[:, l0:l0 + RW])
        nc.vector.tensor_mul(w[:], xsw[:], s128[:, l0:l0 + RW])
        nc.vector.tensor_add(dst[:, cols], u[:], w[:])

    def rope_nt(nt):
        # called on odd nt: ropes token blocks nt-1 and nt together
        b, l0 = nt // 4, (nt % 4 - 1) * NB
        for p in range(2):
            c0 = p * T + b * L + l0
            rope(qT, slice(c0, c0 + RW), l0)
        rope(kT, slice(b * L + l0, b * L + l0 + RW), l0)

    # ---------------- phase 1: QKV projections ----------------------------
    # k and v projections share one [128,128] stationary (cols 0:64 = wk,
    # 64:128 = wv); v comes out feature-major and is PE-transposed into vA.
    with tc.tile_pool(name="xs", bufs=10) as xs, \
         tc.tile_pool(name="vstage", bufs=2) as vs, \
         tc.tile_pool(name="pproj", bufs=2, space="PSUM") as pq, \
         tc.tile_pool(name="pprojkv", bufs=2, space="PSUM") as pkv, \
         tc.tile_pool(name="ptr", bufs=2, space="PSUM") as ptr:
        for nt in range(NT):
            psq = [pq.tile([128, NB], F32, tag=f"psq{i}", name=f"psq{nt}_{i}") for i in range(2)]
            pskv = pkv.tile([128, NB], F32, tag="pskv", name=f"pskv{nt}")
            for kc in range(KC):
                xk = xs.tile([128, NB], BF16, tag="xk")
                nc.sync.dma_start(
                    xk[:], d_xT[kc * 128:(kc + 1) * 128, nt * NB:(nt + 1) * NB])
                st, sp = kc == 0, kc == KC - 1
                for p in range(2):
                    nc.tensor.matmul(
                        psq[p][:], wqT[:, kc * DQ + p * 128: kc * DQ + (p + 1) * 128],
                        xk[:], start=st, stop=sp)
                nc.tensor.matmul(pskv[:], wkvT[:, kc * 128:(kc + 1) * 128], xk[:],
                                 start=st, stop=sp)
            # evictions (DVE copies round fp32 -> bf16)
            for p in range(2):
                nc.vector.tensor_copy(qT[:, p * T + nt * NB: p * T + (nt + 1) * NB],
                                      psq[p][:])
            nc.vector.tensor_copy(kT[0:64, nt * NB:(nt + 1) * NB], pskv[0:64, :])
            nc.vector.tensor_copy(kT[64:128, nt * NB:(nt + 1) * NB], pskv[0:64, :])
            vTf = vs.tile([64, NB], BF16, tag="vTf")
            nc.vector.tensor_copy(vTf[:], pskv[64:128, :])
            for s in range(4):
                ch = nt * 4 + s
                ptv = ptr.tile([128, HD], BF16, tag="ptv", name=f"ptv{nt}_{s}")
                nc.tensor.transpose(ptv[:], vTf[:, s * 128:(s + 1) * 128],
                                    ident[:])
                nc.vector.tensor_copy(vA[:, ch * 65: ch * 65 + 64], ptv[:])
            if nt % 2 == 1:
                rope_nt(nt)

    # ---------------- phase 2+3: attention + out-projection ---------------
    # two-pass attention per (b, ib, head-pair): (1) scores + exp for all key
    # chunks into an SBUF slab, (2) one dense back-to-back AV matmul stream.
    # Pass 2 of head-pair p overlaps pass 1 of head-pair p+1 on the PE.
    with tc.tile_pool(name="epool", bufs=2) as ep, \
         tc.tile_pool(name="opool", bufs=6) as op, \
         tc.tile_pool(name="pst", bufs=2, space="PSUM") as pst, \
         tc.tile_pool(name="pot", bufs=1, space="PSUM") as pot, \
         tc.tile_pool(name="pmisc", bufs=2, space="PSUM") as pmisc:
        for b in range(B):
            for ib in range(L // NB):
                for p in range(2):
                    qcols = p * T + b * L + ib * NB
                    # one [65,512] accumulator per head of the pair (2 banks)
                    ot = [pot.tile([65, NB], F32, tag=f"ot{o}", name=f"ot{b}_{ib}_{p}_{o}")
                          for o in range(2)]
                    njc = 4 * (ib + 1)
                    eslab = ep.tile([128, 16 * 2 * NB], BF16, tag="eslab",
                                    name=f"eslab{b}_{ib}_{p}")
                    for jc in range(njc):
                        # diagonal chunks: columns < c*128 are fully masked ->
                        # never computed, never read (narrowed matmuls)
                        c = jc - 4 * ib
                        n0 = max(c, 0) * 128
                        # even/odd head score matmuls row-tile at base
                        # partitions 0/64 -> shared [128, 2*NB] PSUM tile
                        # (2 banks) -> one batched exp
                        sc = pst.tile([128, 2 * NB], F32, tag="sc")
                        kcols = slice(b * L + jc * 128, b * L + (jc + 1) * 128)
                        nc.tensor.matmul(sc[:, n0:NB], kT[0:64, kcols],
                                         qT[0:64, qcols + n0:qcols + NB],
                                         start=True, stop=True)
                        nc.tensor.matmul(sc[:, NB + n0:2 * NB], kT[64:128, kcols],
                                         qT[64:128, qcols + n0:qcols + NB],
                                         start=True, stop=True)
                        # exp over the full tile: stale/garbage columns are
                        # finite and never consumed downstream
                        e0 = jc * 2 * NB
                        nc.scalar.activation(eslab[:, e0:e0 + 2 * NB], sc[:],
                                             AF.Exp, scale=float(SCALE))
                        if c >= 0:  # diagonal 128-col band: triangle mask
                            nc.vector.tensor_mul(eslab[:, e0 + n0:e0 + n0 + 128],
                                                 eslab[:, e0 + n0:e0 + n0 + 128], tri[:])
                            nc.vector.tensor_mul(eslab[:, e0 + NB + n0:e0 + NB + n0 + 128],
                                                 eslab[:, e0 + NB + n0:e0 + NB + n0 + 128], tri[:])
                    for jc in range(njc):
                        c = jc - 4 * ib
                        n0 = max(c, 0) * 128
                        e0 = jc * 2 * NB
                        ch = b * 16 + jc  # 16 chunks of 128 tokens per b
                        st, sp = jc == 0, jc == njc - 1
                        nc.tensor.matmul(ot[0][:, n0:NB], vA[:, ch * 65: ch * 65 + 65],
                                         eslab[:, e0 + n0:e0 + NB], start=st, stop=sp)
                        nc.tensor.matmul(ot[1][:, n0:NB], vA[:, ch * 65: ch * 65 + 65],
                                         eslab[:, e0 + NB + n0:e0 + 2 * NB], start=st, stop=sp)
                    for o in range(2):
                        rbase = 64 * o
                        # rounding copy PSUM->SBUF so the f32r matmul is legal
                        drow = tpool.tile([1, NB], F32R, tag="drow")
                        nc.vector.tensor_copy(drow[:], ot[o][64:65, :])
                        # stage ot out of PSUM right away so the bank frees
                        # for the next head-pair's AV accumulation
                        ot_sb = tpool.tile([64, NB], F32, tag="ot_sb")
                        nc.vector.tensor_copy(ot_sb[:], ot[o][0:64, :])
                        bc_ps = pmisc.tile([64, NB], F32, tag="mps", name=f"bc{b}_{ib}_{p}_{o}")
                        nc.tensor.matmul(bc_ps[:], ones2[0:1, 64:128], drow[:],
                                         start=True, stop=True)
                        denb = tpool.tile([64, NB], F32, tag="denb")
                        nc.vector.reciprocal_approx_fast(denb[:], bc_ps[:])
                        nc.vector.tensor_mul(
                            atP[p][rbase:rbase + 64, b * L + ib * NB: b * L + (ib + 1) * NB],
                            ot_sb[:], denb[:])
                # out-projection for this (b, ib) token block
                nt = b * 4 + ib
                for mc in range(16):
                    po = pmisc.tile([128, NB], F32, tag="mps", name=f"po{nt}_{mc}")
                    nc.tensor.matmul(po[:], woT[:, 0 * D + mc * 128: 0 * D + (mc + 1) * 128],
                                     atP[0][:, nt * NB:(nt + 1) * NB],
                                     start=True, stop=False)
                    nc.tensor.matmul(po[:], woT[:, 1 * D + mc * 128: 1 * D + (mc + 1) * 128],
                                     atP[1][:, nt * NB:(nt + 1) * NB],
                                     start=False, stop=True)
                    osb = op.tile([128, NB], BF16, tag="osb")
                    nc.vector.tensor_copy(osb[:], po[:])
                    nc.gpsimd.dma_start(
                        d_out[mc * 128:(mc + 1) * 128, nt * NB:(nt + 1) * NB], osb[:])

    if dump is not None:
        with tc.tile_pool(name="dumpp", bufs=2) as dp:
            for name, t in [("qT", qT), ("kT", kT), ("vA", vA),
                            ("atP0", atP[0]), ("atP1", atP[1])]:
                if name not in dump:
                    continue
                stage = dp.tile(list(t.shape), F32, tag="dstage",
                                name=f"dump_{name}")
                nc.vector.tensor_copy(stage[:], t[:])
                nc.sync.dma_start(dump[name][:], stage[:])


def _deinterleave_rows(w):
    # [H*64, D] -> per-head rows reordered to [even dims | odd dims]
    h = w.shape[0] // HD
    out = np.empty_like(w)
    perm = np.concatenate([np.arange(0, HD, 2), np.arange(1, HD, 2)])
    for i in range(h):
        out[i * HD:(i + 1) * HD] = w[i * HD:(i + 1) * HD][perm]
    return out


def _prep_inputs(x, pos_ids, wq, wk, wv, wo):
    xT = np.ascontiguousarray(x.reshape(T, D).T).astype(ml_dtypes.bfloat16)
    half = HD // 2
    invf = (1.0 / (ROPE_BASE ** (np.arange(half, dtype=np.float32) / half)))
    freqs = pos_ids.astype(np.float32)[None, :] * invf[:, None]  # [32, L]
    cos = np.cos(freqs)
    sin = np.sin(freqs)
    cosb = np.tile(cos, (4, 1))
    sinb = np.tile(sin, (4, 1))
    sinb[0:32] *= -1.0   # even-dim bands get -sin (y_e = x_e*c - x_o*s)
    sinb[64:96] *= -1.0
    # causal triangle for any diagonal 128-col band: query col q attends
    # key row r iff r <= q
    tri = (np.arange(128)[:, None] <= np.arange(128)[None, :])
    onesv = np.zeros((128, 32 * 65), np.float32)
    onesv[:, 64::65] = 1.0
    ones2 = np.concatenate([np.zeros((1, 64), np.float32),
                            np.ones((1, 64), np.float32)], axis=1)
    ones64 = np.ones((1, 64), np.float32)
    ident = np.eye(64, dtype=np.float32)
    in_maps = []
    for c in range(NCORES):
        wq_c = _deinterleave_rows(wq[c * DQ:(c + 1) * DQ])
        wk_c = _deinterleave_rows(wk[c * HD:(c + 1) * HD])
        wv_c = wv[c * HD:(c + 1) * HD]
        wkv_c = np.concatenate([wk_c.T, wv_c.T], axis=1)  # [D, 128]
        wo_c = wo[:, c * DQ:(c + 1) * DQ]
        in_maps.append({
            "xT": xT,
            "wq_t": np.ascontiguousarray(wq_c.T).astype(ml_dtypes.bfloat16),
            "wkv_t": np.ascontiguousarray(wkv_c).astype(ml_dtypes.bfloat16),
            "wo_t": np.ascontiguousarray(wo_c.T).astype(ml_dtypes.bfloat16),
            "cosb": cosb.astype(ml_dtypes.bfloat16),
            "sinb": sinb.astype(ml_dtypes.bfloat16),
            "tri": tri.astype(ml_dtypes.bfloat16),
            "onesv": onesv.astype(ml_dtypes.bfloat16),
            "ones2": ones2,
            "ident": ident.astype(ml_dtypes.bfloat16),
        })
    return in_maps


def kernel(x, pos_ids, wq, wk, wv, wo, _trace=False):
    x = np.asarray(x)
    if "nc" not in _CACHE:
        _CACHE["nc"] = _build_module()
    nc = _CACHE["nc"]
    in_maps = _prep_inputs(np.asarray(x, np.float32), np.asarray(pos_ids),
                           np.asarray(wq, np.float32), np.asarray(wk, np.float32),
                           np.asarray(wv, np.float32), np.asarray(wo, np.float32))
    res = run_bass_kernel_spmd(nc, in_maps, core_ids=list(range(NCORES)),
                               trace=_trace)
    _CACHE["last_results"] = res
    acc = np.zeros((D, T), np.float32)
    for r in res.results:
        acc += r["outT"].astype(np.float32)
    return np.ascontiguousarray(acc.T).reshape(B, L, D)
